# revision 7
# baseline (speedup 1.0000x reference)
"""Trainium2 Bass kernel for nn_CoupledModel (24 tiny MLPs + softmax-mixture coupling).

Strategy (per core, data-parallel over 8 cores):
  - Layout: features on partitions, samples on free dim. 24 nets parity-reordered
    (evens first) and split into 6 groups of 4 nets (32 features/group).
  - L1/L2/L3 as tile_position-packed small matmuls on TensorE (block-diag weights).
  - tanh(+bias) fused on ScalarE reading PSUM directly.
  - Coupling uses the free-energy identity:
        output = -kb*T * ( ln S + sum_m w_m * h_m ),
        h_m = 4*q_m^2, q = odds^2, S = sum exp(-10*E/T + 10*q - h)
    so no per-element softmax normalization / log(w) is needed.
  - Ln (different ACT table set than Tanh/Exp) is batched into one tail op.
"""
import sys
import numpy as np

sys.path.insert(0, "/opt/trn_rl_repo")
sys.path.insert(0, "/opt/trn_rl_repo/concourse")

N_TOTAL = 262144
N_CORES = 8
NC = N_TOTAL // N_CORES          # 32768 samples per core
CHUNK = 512
SUPER_CHUNKS = 8                 # chunks per super
N_SUPER = NC // (CHUNK * SUPER_CHUNKS)   # 8
N_CHUNKS = NC // CHUNK           # 64

_PERM = np.array([2 * r for r in range(12)] + [2 * r + 1 for r in range(12)])


def _prep_consts(W1, b1, W2, b2, W3):
    W1r, b1r = W1[_PERM], b1[_PERM]
    W2r, b2r = W2[_PERM], b2[_PERM]
    W3r = W3[_PERM]
    f32 = np.float32

    w1a = np.zeros((2, 128), f32)
    w1b = np.zeros((2, 64), f32)
    for g in range(4):
        for j in range(4):
            # w1a[d, 32g+8j+u] = W1r[4g+j, u, d]
            w1a[:, 32 * g + 8 * j:32 * g + 8 * j + 8] = W1r[4 * g + j].T
    for k in range(2):
        for j in range(4):
            w1b[:, 32 * k + 8 * j:32 * k + 8 * j + 8] = W1r[16 + 4 * k + j].T

    w2a = np.zeros((128, 128), f32)
    for g in range(4):
        for j in range(4):
            base = 32 * g + 8 * j
            # w2a[base+w, base+v] = W2r[net, v, w]
            w2a[base:base + 8, base:base + 8] = W2r[4 * g + j].T
    w2b = np.zeros((128, 32), f32)
    for half in (0, 64):
        for k in range(2):
            for j in range(4):
                r0 = half + 32 * k + 8 * j
                w2b[r0:r0 + 8, 8 * j:8 * j + 8] = W2r[16 + 4 * k + j].T

    w3a = np.zeros((128, 16), f32)
    for g in range(4):
        for j in range(4):
            w3a[32 * g + 8 * j:32 * g + 8 * j + 8, 4 * g + j] = W3r[4 * g + j, 0]
    w3b = np.zeros((128, 8), f32)
    for half in (0, 64):
        for k in range(2):
            for j in range(4):
                r0 = half + 32 * k + 8 * j
                w3b[r0:r0 + 8, 4 * k + j] = W3r[16 + 4 * k + j, 0]

    b1a = np.zeros((128,), f32)
    b2a = np.zeros((128,), f32)
    for g in range(4):
        for j in range(4):
            b1a[32 * g + 8 * j:32 * g + 8 * j + 8] = b1r[4 * g + j]
            b2a[32 * g + 8 * j:32 * g + 8 * j + 8] = b2r[4 * g + j]
    b1b = np.zeros((128,), f32)
    b2b = np.zeros((128,), f32)
    for half in (0, 64):
        for k in range(2):
            for j in range(4):
                r0 = half + 32 * k + 8 * j
                b1b[r0:r0 + 8] = b1r[16 + 4 * k + j]
                b2b[r0:r0 + 8] = b2r[16 + 4 * k + j]

    red = np.zeros((96, 8), f32)
    bc = np.zeros((8, 96), f32)
    for p in range(96):
        red[p, p // 12] = 1.0
        bc[p // 12, p] = 10.0  # folds the 1/(kb*T) = 10/T factor into the bcast

    return {
        "w1a": w1a, "w1b": w1b, "w2a": w2a, "w2b": w2b,
        "w3a": w3a, "w3b": w3b,
        "b1a": b1a, "b1b": b1b, "b2a": b2a, "b2b": b2b,
        "red": red, "bc": bc,
    }


def build_nc():
    import concourse.bacc as bacc
    import concourse.bass as bass
    import concourse.mybir as mybir
    from concourse import tile

    F32 = mybir.dt.float32
    AF = mybir.ActivationFunctionType
    ALU = mybir.AluOpType

    nc = bacc.Bacc(None, target_bir_lowering=False)

    x_d = nc.dram_tensor("x", [NC, 2], F32, kind="ExternalInput")
    cd = {}
    for name, shape in [
        ("w1a", [2, 128]), ("w1b", [2, 64]), ("w2a", [128, 128]),
        ("w2b", [128, 32]), ("w3a", [128, 16]), ("w3b", [128, 8]),
        ("b1a", [128]), ("b1b", [128]), ("b2a", [128]), ("b2b", [128]),
        ("red", [96, 8]), ("bc", [8, 96]),
    ]:
        cd[name] = nc.dram_tensor(name, shape, F32, kind="ExternalInput")
    out_d = nc.dram_tensor("out", [NC, 1], F32, kind="ExternalOutput")
    subT_d = nc.dram_tensor("subT", [24, NC], F32, kind="ExternalOutput")

    with tile.TileContext(nc) as tc:
        with (
            tc.tile_pool(name="cst", bufs=1) as cst,
            tc.tile_pool(name="xt", bufs=4) as xtp,
            tc.tile_pool(name="ha", bufs=4) as hap,
            tc.tile_pool(name="hb", bufs=4) as hbp,
            tc.tile_pool(name="eo", bufs=2) as eop,
            tc.tile_pool(name="cw", bufs=2) as cwp,
            tc.tile_pool(name="gl", bufs=1) as glp,
            tc.tile_pool(name="pp", bufs=6, space=bass.MemorySpace.PSUM) as ppp,
            tc.tile_pool(name="bd", bufs=2, space=bass.MemorySpace.PSUM) as bdp,
        ):
            # ---- constants
            w1t = cst.tile([34, 128], F32)
            nc.sync.dma_start(w1t[0:2, :], cd["w1a"][:])
            nc.sync.dma_start(w1t[32:34, 0:64], cd["w1b"][:])
            w2at = cst.tile([128, 128], F32)
            nc.sync.dma_start(w2at[:], cd["w2a"][:])
            w2bt = cst.tile([128, 32], F32)
            nc.sync.dma_start(w2bt[:], cd["w2b"][:])
            w3at = cst.tile([128, 16], F32)
            nc.sync.dma_start(w3at[:], cd["w3a"][:])
            w3bt = cst.tile([128, 8], F32)
            nc.sync.dma_start(w3bt[:], cd["w3b"][:])
            bt = {}
            for nm in ("b1a", "b1b", "b2a", "b2b"):
                bt[nm] = cst.tile([128, 1], F32, name="b_" + nm, tag=nm)
                nc.sync.dma_start(bt[nm][:], cd[nm][:].rearrange("(m o) -> m o", o=1))
            redt = cst.tile([96, 8], F32)
            nc.sync.dma_start(redt[:], cd["red"][:])
            bct = cst.tile([8, 96], F32)
            nc.sync.dma_start(bct[:], cd["bc"][:])

            # ---- global tiles
            Tall = glp.tile([64, 512], F32)
            nc.sync.dma_start(
                Tall[:], x_d[:, 1:2].rearrange("(c f) o -> c (f o)", f=512))
            rTall = glp.tile([64, 512], F32)
            scr = glp.tile([64, 512], F32)
            nc.vector.reciprocal_approx_accurate(rTall[:], Tall[:], scr[:])
            Sall = glp.tile([64, 512], F32)
            Rall = glp.tile([64, 512], F32)

            for s in range(N_SUPER):
                E_sb = eop.tile([96, 512], F32, tag="E")
                O_sb = eop.tile([96, 512], F32, tag="O")
                h2a_keep = {}
                for p in range(4):
                    pB = bdp.tile([128, 512], F32, tag="bd")
                    pD = bdp.tile([128, 512], F32, tag="bd")
                    for ci in range(2):
                        lc = 2 * p + ci
                        cc = SUPER_CHUNKS * s + lc
                        n0 = cc * CHUNK
                        xt = xtp.tile([34, 512], F32, tag="xt")
                        xsl = x_d[n0:n0 + CHUNK, :].rearrange("n d -> d n")
                        nc.sync.dma_start(xt[0:2, :], xsl)
                        nc.sync.dma_start(xt[32:34, :], xsl)
                        # L1: 4 tiles into pA, 2 into pB
                        pA = ppp.tile([128, 512], F32, tag="pp")
                        for g in range(4):
                            nc.tensor.matmul(
                                pA[32 * g:32 * g + 32, :],
                                w1t[0:2, 32 * g:32 * g + 32],
                                xt[0:2, :], tile_position=(0, 32 * g))
                        for k in range(2):
                            col = 2 * ci + k
                            nc.tensor.matmul(
                                pB[32 * col:32 * col + 32, :],
                                w1t[32:34, 32 * k:32 * k + 32],
                                xt[32:34, :], tile_position=(32, 32 * col))
                        h1a = hap.tile([128, 512], F32, tag="h1a")
                        nc.scalar.activation(h1a[:], pA[:], AF.Tanh, bias=bt["b1a"][:])
                        # L2a: 4 diag tiles
                        pC = ppp.tile([128, 512], F32, tag="pp")
                        for g in range(4):
                            nc.tensor.matmul(
                                pC[32 * g:32 * g + 32, :],
                                w2at[32 * g:32 * g + 32, 32 * g:32 * g + 32],
                                h1a[32 * g:32 * g + 32, :],
                                tile_position=(32 * g, 32 * g))
                        h2a = hap.tile([128, 512], F32, tag="h2a")
                        nc.scalar.activation(h2a[:], pC[:], AF.Tanh, bias=bt["b2a"][:])
                        h2a_keep[lc] = h2a
                    # pair boundary: L1b tanh, L2b, L2b tanh
                    h1b = hbp.tile([128, 512], F32, tag="h1b")
                    nc.scalar.activation(h1b[:], pB[:], AF.Tanh, bias=bt["b1b"][:])
                    # L2b: even chunk (ci=0) reads h1b[0:64] -> pD[64:128]
                    #      odd  chunk (ci=1) reads h1b[64:128] -> pD[0:64]
                    nc.tensor.matmul(pD[64:96, :], w2bt[0:32, :], h1b[0:32, :],
                                     tile_position=(0, 64))
                    nc.tensor.matmul(pD[96:128, :], w2bt[32:64, :], h1b[32:64, :],
                                     tile_position=(32, 96))
                    nc.tensor.matmul(pD[0:32, :], w2bt[64:96, :], h1b[64:96, :],
                                     tile_position=(64, 0))
                    nc.tensor.matmul(pD[32:64, :], w2bt[96:128, :], h1b[96:128, :],
                                     tile_position=(96, 32))
                    h2b = hbp.tile([128, 512], F32, tag="h2b")
                    nc.scalar.activation(h2b[:], pD[:], AF.Tanh, bias=bt["b2b"][:])
                    # L3 per chunk of the pair
                    for ci in range(2):
                        lc = 2 * p + ci
                        pE = ppp.tile([128, 512], F32, tag="pp")
                        nc.tensor.matmul(pE[0:16, :], w3at[:, :],
                                         h2a_keep[lc][:, :], tile_position=(0, 0))
                        if ci == 0:
                            nc.tensor.matmul(pE[32:40, :], w3bt[64:128, :],
                                             h2b[64:128, :], tile_position=(64, 32))
                        else:
                            nc.tensor.matmul(pE[32:40, :], w3bt[0:64, :],
                                             h2b[0:64, :], tile_position=(0, 32))
                        # pack into E/O: DVE copy psum->sbuf, then shift DMAs
                        r0 = 12 * lc
                        stg = xtp.tile([40, 512], F32, tag="stg")
                        nc.vector.tensor_copy(stg[:], pE[0:40, :])
                        nc.sync.dma_start(E_sb[r0:r0 + 12, :], stg[0:12, :])
                        nc.sync.dma_start(O_sb[r0:r0 + 4, :], stg[12:16, :])
                        nc.sync.dma_start(O_sb[r0 + 4:r0 + 12, :], stg[32:40, :])
                        # sub output (interleave evens/odds via row-pair view)
                        n0 = (SUPER_CHUNKS * s + lc) * CHUNK
                        sub2 = subT_d[:].rearrange("(a b) n -> a b n", b=2)
                        nc.sync.dma_start(sub2[:, 0, n0:n0 + CHUNK],
                                          E_sb[r0:r0 + 12, :])
                        nc.sync.dma_start(sub2[:, 1, n0:n0 + CHUNK],
                                          O_sb[r0:r0 + 12, :])

                # ---- coupling (main phase, per super)
                rtp = cwp.tile([8, 512], F32, tag="rtp")
                nc.sync.dma_start(rtp[:], rTall[8 * s:8 * s + 8, :])
                rTb = bdp.tile([128, 512], F32, tag="bd")
                nc.tensor.matmul(rTb[0:96, :], bct[:, :], rtp[:, :],
                                 tile_position=(0, 0))
                q = cwp.tile([96, 512], F32, tag="q")
                nc.vector.tensor_tensor(q[:], O_sb[:], O_sb[:], ALU.mult)
                h4 = cwp.tile([96, 512], F32, tag="h4")
                nc.vector.scalar_tensor_tensor(h4[:], q[:], 4.0, q[:],
                                               ALU.mult, ALU.mult)
                u = cwp.tile([96, 512], F32, tag="u")
                nc.vector.tensor_tensor(u[:], E_sb[:], rTb[0:96, :], ALU.mult)
                t1 = cwp.tile([96, 512], F32, tag="t1")
                nc.vector.scalar_tensor_tensor(t1[:], q[:], 10.0, h4[:],
                                               ALU.mult, ALU.subtract)
                garg = cwp.tile([96, 512], F32, tag="garg")
                nc.vector.scalar_tensor_tensor(garg[:], u[:], -1.0, t1[:],
                                               ALU.mult, ALU.add)
                logits = cwp.tile([96, 512], F32, tag="logits")
                nc.scalar.activation(logits[:], garg[:], AF.Exp)
                ph = cwp.tile([96, 512], F32, tag="ph")
                nc.vector.tensor_tensor(ph[:], logits[:], h4[:], ALU.mult)
                Fp = bdp.tile([128, 512], F32, tag="bd")
                nc.tensor.matmul(Fp[0:8, :], redt[:, :], logits[:, :],
                                 tile_position=(0, 0))
                nc.tensor.matmul(Fp[32:40, :], redt[:, :], ph[:, :],
                                 tile_position=(0, 32))
                SR_sb = cwp.tile([40, 512], F32, tag="SR")
                nc.vector.tensor_copy(SR_sb[:], Fp[0:40, :])
                nc.sync.dma_start(Sall[8 * s:8 * s + 8, :], SR_sb[0:8, :])
                nc.sync.dma_start(Rall[8 * s:8 * s + 8, :], SR_sb[32:40, :])

            # ---- tail (one ACT table switch for Ln)
            rSall = glp.tile([64, 512], F32)
            scr2 = glp.tile([64, 512], F32)
            nc.vector.reciprocal_approx_accurate(rSall[:], Sall[:], scr2[:])
            lnS = glp.tile([64, 512], F32)
            nc.scalar.activation(lnS[:], Sall[:], AF.Ln)
            mall = glp.tile([64, 512], F32)
            nc.vector.tensor_tensor(mall[:], Rall[:], rSall[:], ALU.mult)
            tadd = glp.tile([64, 512], F32)
            nc.vector.tensor_tensor(tadd[:], mall[:], lnS[:], ALU.add)
            out_all = glp.tile([64, 512], F32)
            nc.vector.scalar_tensor_tensor(out_all[:], tadd[:], -0.1, Tall[:],
                                           ALU.mult, ALU.mult)
            nc.sync.dma_start(
                out_d[:].rearrange("(c f) o -> c (f o)", f=512), out_all[:])

    nc.compile()
    return nc


_NC_CACHE = None


def _get_nc():
    global _NC_CACHE
    if _NC_CACHE is None:
        _NC_CACHE = build_nc()
    return _NC_CACHE


def make_in_maps(x, W1, b1, W2, b2, W3):
    consts = _prep_consts(
        np.asarray(W1, np.float32), np.asarray(b1, np.float32),
        np.asarray(W2, np.float32), np.asarray(b2, np.float32),
        np.asarray(W3, np.float32))
    x = np.ascontiguousarray(np.asarray(x, np.float32))
    in_maps = []
    for c in range(N_CORES):
        m = {"x": x[c * NC:(c + 1) * NC]}
        m.update(consts)
        in_maps.append(m)
    return in_maps


def kernel(x, W1, b1, W2, b2, W3):
    from concourse.bass_utils import run_bass_kernel_spmd

    nc = _get_nc()
    in_maps = make_in_maps(x, W1, b1, W2, b2, W3)
    res = run_bass_kernel_spmd(nc, in_maps, list(range(N_CORES)))
    outs = []
    subs = []
    for c in range(N_CORES):
        outs.append(res.results[c]["out"])
        subs.append(np.ascontiguousarray(res.results[c]["subT"].T))
    output = np.concatenate(outs, axis=0).astype(np.float32)
    sub = np.concatenate(subs, axis=0).astype(np.float32)
    return output, sub
